# revision 9
# baseline (speedup 1.0000x reference)
"""Conv2d 3x3 VALID via 1D Winograd F(6,3) along H, batch-sharded on 8 cores.

Problem: input [32,128,64,64] f32, weights [256,128,3,3] f32 ->
output [32,256,62,62] f32 (stride 1, no padding).

Scheme (fp16 end to end on device; error ~3e-3 vs f32 reference):
  - Host: Cook-Toom F(6,3) input transform along H with points
    (0,1,-1,2,-2,1/2,-1/2,inf): V[k=0..7, t=0..9, x] per (b, ci), plus an
    F(2,3) tail pair for output rows 60-61. Weight transform G w per
    (k, kw, Cout-half), shipped pre-transposed as lhsT [ci, co].
  - Device (per core, 4 images): M[k][co, t, x] = sum_kw U[k,kw]^T V[k, t, x+kw]
    PSUM-accumulated over kw (f32), 310-col matmuls (5 H-tiles per block).
    Each LDWEIGHTS is shared by the 4 images' matmuls. PSUM is evacuated
    as fp16 by DVE/ACT copies into a staging tile, DMA'd to DRAM.
  - Host: inverse transform Y = A^T M (tiny 6x8 combine) in f32.
"""

import numpy as np

import concourse.bass as bass
import concourse.mybir as mybir
import concourse.tile as tile
from concourse import bacc
from concourse.bass_utils import run_bass_kernel_spmd

F32 = mybir.dt.float32
FP16 = mybir.dt.float16

B, CIN, H, W = 32, 128, 64, 64
COUT, KH, KW = 256, 3, 3
OH, OW = H - KH + 1, W - KW + 1  # 62, 62
N_CORES = 8
BL = B // N_CORES  # 4 images per core

M_TILE = 6          # F(6,3): 6 output rows per tile
NK = M_TILE + 2     # 8 winograd components
NT = 60 // M_TILE   # 10 H-tiles (output rows 0..59)
TPB = 5             # H-tiles per block
NBLK = NT // TPB    # 2 blocks per (image, half)
S = TPB * OW        # 310 matmul columns per block
NKT = 4             # F(2,3) tail components (output rows 60-61)

# ---------------------------------------------------------------------------
# Cook-Toom transform matrices
# ---------------------------------------------------------------------------


def _derive(m, points):
    """F(m,3) Cook-Toom matrices for given finite points (+infinity).
    Returns At [m,n], G [n,3], Bt [n,n] (f64), n = m+2."""
    from fractions import Fraction

    r = 3
    n = m + r - 1
    pts = [Fraction(p) for p in points]
    At = [[float(a**j) for a in pts] + ([1.0] if j == m - 1 else [0.0])
          for j in range(m)]
    G = []
    for i, a in enumerate(pts):
        N = Fraction(1)
        for j, b in enumerate(pts):
            if i != j:
                N *= a - b
        G.append([float((a**s) / N) for s in range(r)])
    G.append([0.0] * (r - 1) + [1.0])
    A = np.array(At)
    Gf = np.array(G)
    Mm = np.zeros((r * m, n))
    for s in range(r):
        for j in range(m):
            Mm[s * m + j] = A[j] * Gf[:, s]
    Bt = np.zeros((n, n))
    for t in range(n):
        rhs = np.array(
            [1.0 if (t - s) == j else 0.0 for s in range(r) for j in range(m)]
        )
        sol, *_ = np.linalg.lstsq(Mm, rhs, rcond=None)
        assert np.abs(Mm @ sol - rhs).max() < 1e-9
        Bt[:, t] = sol
    return A, Gf, Bt


_PTS6 = [0, 1, -1, 2, -2, 0.5, -0.5]
A6, G6, B6 = _derive(M_TILE, _PTS6)
A2 = np.array([[1.0, 1, 1, 0], [0, 1, -1, -1]])
G2 = np.array([[1.0, 0, 0], [0.5, 0.5, 0.5], [0.5, -0.5, 0.5], [0, 0, 1]])
B2 = np.array([
    [1.0, 0, -1, 0],
    [0, 1, 1, 0],
    [0, -1, 1, 0],
    [0, 1, 0, -1],
])  # Bt[k, r]: V_k = sum_r Bt[k,r] d_r


# ---------------------------------------------------------------------------
# Device kernel
# ---------------------------------------------------------------------------


def _conv_body(nc, tc, m_d, mt_d, v_d, vt_d, w_d, wt_d):
    with (
        tc.tile_pool(name="vin", bufs=1) as v_pool,
        tc.tile_pool(name="win", bufs=1) as w_pool,
        tc.tile_pool(name="psum", bufs=8, space=bass.MemorySpace.PSUM) as ps_pool,
        tc.tile_pool(name="stage", bufs=2) as st_pool,
        tc.tile_pool(name="lstage", bufs=2) as lst_pool,
        tc.tile_pool(name="tstage", bufs=2) as tst_pool,
    ):
        w_sb = w_pool.tile([128, 2, NK, KW, 128], FP16, name="w_sb")
        wt_sb = w_pool.tile([128, 2, NKT, KW, 128], FP16, name="wt_sb")
        v_tiles = [v_pool.tile([128, NBLK, NK, TPB, W], FP16, name=f"v{b}") for b in range(BL)]
        vt_sb = v_pool.tile([128, BL, NKT, W], FP16, name="vt_sb")

        # Warm up the PE HAM clock gate during the initial DMA wait: dummy
        # matmuls on a zeroed tile so the real stream starts at full clock.
        # Emitted first so no engine queue work delays the PE.
        scratch = w_pool.tile([128, 128], FP16, name="scratch")
        nc.gpsimd.memset(scratch, 0)
        ps_warm = ps_pool.tile([128, 512], F32, tag="ps", name="ps")
        for _ in range(22):
            nc.tensor.matmul(ps_warm[:, :128], scratch, scratch,
                             start=True, stop=True)

        # Startup DMA: k-major chunks so comp k's data for ALL images lands
        # before comp k+2's, issued across four queues in parallel (issue
        # cost is ~0.7us per DMA per queue).
        for k0 in range(0, NK, 2):
            nc.sync.dma_start(
                out=w_sb[:, 0, k0 : k0 + 2], in_=w_d[:, 0, k0 : k0 + 2]
            )
            nc.sync.dma_start(
                out=v_tiles[3][:, 0, k0 : k0 + 2], in_=v_d[3, :, 0, k0 : k0 + 2]
            )
            nc.gpsimd.dma_start(
                out=v_tiles[0][:, 0, k0 : k0 + 2], in_=v_d[0, :, 0, k0 : k0 + 2]
            )
            nc.scalar.dma_start(
                out=v_tiles[1][:, 0, k0 : k0 + 2], in_=v_d[1, :, 0, k0 : k0 + 2]
            )
            nc.gpsimd.dma_start(
                out=v_tiles[2][:, 0, k0 : k0 + 2], in_=v_d[2, :, 0, k0 : k0 + 2]
            )
        # remainder: weights h=1 (needed at ~16us), block 1, tails
        nc.sync.dma_start(out=w_sb[:, 1], in_=w_d[:, 1])
        nc.gpsimd.dma_start(out=v_tiles[0][:, 1], in_=v_d[0, :, 1])
        nc.scalar.dma_start(out=v_tiles[1][:, 1], in_=v_d[1, :, 1])
        nc.gpsimd.dma_start(out=v_tiles[2][:, 1], in_=v_d[2, :, 1])
        nc.sync.dma_start(out=v_tiles[3][:, 1], in_=v_d[3, :, 1])
        nc.sync.dma_start(out=wt_sb, in_=wt_d)
        nc.gpsimd.dma_start(out=vt_sb, in_=vt_d)

        def evac(idx, dst, src):
            # alternate PSUM->SBUF fp16 copies between DVE and ACT
            if idx % 2 == 0:
                nc.vector.tensor_copy(dst, src)
            else:
                nc.scalar.activation(dst, src,
                                     mybir.ActivationFunctionType.Copy)

        # (blk, h) order: block-0 V feeds both Cout halves before block-1's
        # data is needed, doubling the DMA deadline for the block-1 stream.
        # Staging is one super-tile per (blk, h) -> one merged out-DMA; the
        # last two comps live in a separate small tile so the final block's
        # early DMA of k0:6 never blocks the k6:7 evacuations.
        NKM = NK - 2
        for blk in range(NBLK):
            for h in range(2):
                st = st_pool.tile([128, BL, NKM, S], FP16, tag="st", name="st")
                lst = lst_pool.tile([128, BL, 2, S], FP16, tag="lst", name="lst")
                last = blk == NBLK - 1 and h == 1
                pss = {}
                for k in range(NK):
                    for kw in range(KW):
                        lhsT = w_sb[:, h, k, kw, :]
                        for b in range(BL):
                            if kw == 0:
                                pss[b] = ps_pool.tile([128, 512], F32, tag="ps", name=f"ps{b}")
                            nc.tensor.matmul(
                                pss[b][:, :S].rearrange("p (t x) -> p t x", x=OW),
                                lhsT,
                                v_tiles[b][:, blk, k, :, kw : kw + OW],
                                start=(kw == 0),
                                stop=(kw == KW - 1),
                            )
                    for b in range(BL):
                        dst = st[:, b, k, :] if k < NKM else lst[:, b, k - NKM, :]
                        evac(k * BL + b, dst, pss[b][:, :S])
                    if last and k == NKM - 1:
                        nc.gpsimd.dma_start(out=m_d[h, blk, :, :, :NKM], in_=st)
                if not last:
                    nc.gpsimd.dma_start(out=m_d[h, blk, :, :, :NKM], in_=st)
                nc.gpsimd.dma_start(out=m_d[h, blk, :, :, NKM:], in_=lst)
        # F(2,3) tails (output rows 60-61), both halves: small compute that
        # overlaps the final block's outbound DMA drain
        for h in range(2):
            tst = tst_pool.tile([128, BL, NKT, OW], FP16, tag="tst", name="tst")
            tps = {}
            for k in range(NKT):
                for kw in range(KW):
                    lhsT = wt_sb[:, h, k, kw, :]
                    for b in range(BL):
                        if kw == 0:
                            tps[b] = ps_pool.tile([128, 512], F32, tag="ps", name=f"tps{b}")
                        nc.tensor.matmul(
                            tps[b][:, :OW],
                            lhsT,
                            vt_sb[:, b, k, kw : kw + OW],
                            start=(kw == 0),
                            stop=(kw == KW - 1),
                        )
                for b in range(BL):
                    evac(k * BL + b, tst[:, b, k, :], tps[b][:, :OW])
            nc.gpsimd.dma_start(out=mt_d[h], in_=tst)


def build_module():
    nc = bacc.Bacc(
        "TRN2", target_bir_lowering=False, debug=False, num_devices=N_CORES
    )
    v_d = nc.dram_tensor(
        "v_in", [BL, CIN, NBLK, NK, TPB, W], FP16, kind="ExternalInput"
    ).ap()
    vt_d = nc.dram_tensor(
        "vt_in", [CIN, BL, NKT, W], FP16, kind="ExternalInput"
    ).ap()
    w_d = nc.dram_tensor(
        "w_t", [CIN, 2, NK, KW, 128], FP16, kind="ExternalInput"
    ).ap()
    wt_d = nc.dram_tensor(
        "wt_t", [CIN, 2, NKT, KW, 128], FP16, kind="ExternalInput"
    ).ap()
    m_d = nc.dram_tensor(
        "m_out", [2, NBLK, 128, BL, NK, S], FP16, kind="ExternalOutput"
    ).ap()
    mt_d = nc.dram_tensor(
        "mt_out", [2, 128, BL, NKT, OW], FP16, kind="ExternalOutput"
    ).ap()
    with tile.TileContext(nc) as tc:
        _conv_body(nc, tc, m_d, mt_d, v_d, vt_d, w_d, wt_d)
    nc.compile()
    return nc


_NC_CACHE = {}


def _get_module():
    if "nc" not in _NC_CACHE:
        _NC_CACHE["nc"] = build_module()
    return _NC_CACHE["nc"]


# ---------------------------------------------------------------------------
# Host transforms
# ---------------------------------------------------------------------------


def _host_transforms(input_image: np.ndarray, weights: np.ndarray):
    x = input_image.astype(np.float32)
    # F(6,3) H-transform: windows of 8 rows at stride 6 -> [B,C,NT,8,W]
    win = np.lib.stride_tricks.sliding_window_view(x, NK, axis=2)[:, :, ::M_TILE]
    win = win[:, :, :NT]  # [B, C, NT, W, 8] (window axis appended last)
    B6f = B6.astype(np.float32)
    V = np.einsum("kr,bctwr->bcktw", B6f, win, optimize=True)
    # -> [B, C, NK, NT, W] -> blocks [B, C, NBLK, NK, TPB, W]
    V = V.reshape(B, CIN, NK, NBLK, TPB, W).transpose(0, 1, 3, 2, 4, 5)
    V = np.ascontiguousarray(V, dtype=np.float16)

    # F(2,3) tail on input rows 60..63 (output rows 60-61)
    d = x[:, :, 60:64]  # [B, C, 4, W]
    B2f = B2.astype(np.float32)
    Vt = np.einsum("kr,bcrw->bckw", B2f, d, optimize=True).astype(np.float16)

    wf = weights.astype(np.float32)  # [co, ci, kh, kw]
    U = np.einsum("kr,ocrw->cwko", G6.astype(np.float32), wf, optimize=True)
    # U [ci, kw, k, co] -> [ci, h, k, kw, co']
    U = U.reshape(CIN, KW, NK, 2, 128).transpose(0, 3, 2, 1, 4)
    U = np.ascontiguousarray(U, dtype=np.float16)
    Ut = np.einsum("kr,ocrw->cwko", G2.astype(np.float32), wf, optimize=True)
    Ut = Ut.reshape(CIN, KW, NKT, 2, 128).transpose(0, 3, 2, 1, 4)
    Ut = np.ascontiguousarray(Ut, dtype=np.float16)
    return V, Vt, U, Ut


def _host_combine(m_list, mt_list):
    """m: per-core [2, NBLK, 128, BL, NK, S] fp16; mt: [2, 128, BL, NKT, OW].
    Returns [B, COUT, OH, OW] f32."""
    out = np.empty((B, COUT, OH, OW), np.float32)
    A6f = A6.astype(np.float32)
    A2f = A2.astype(np.float32)
    for i, (m, mt) in enumerate(zip(m_list, mt_list)):
        # [2,NBLK,128,BL,NK,S] -> [BL,2,128,NK,NBLK,TPB,OW]
        mm = m.astype(np.float32).reshape(2, NBLK, 128, BL, NK, TPB, OW)
        mm = mm.transpose(3, 0, 2, 4, 1, 5, 6).reshape(BL, 2, 128, NK, NT, OW)
        y = np.einsum("jk,bhoktx->bhotjx", A6f, mm, optimize=True)
        y = y.reshape(BL, 2, 128, NT * M_TILE, OW).reshape(BL, COUT, 60, OW)
        sl = out[i * BL : (i + 1) * BL]
        sl[:, :, :60] = y
        mtf = mt.astype(np.float32)  # [2, 128, BL, NKT, OW]
        yt = np.einsum("jk,hobkx->bhojx", A2f, mtf, optimize=True)
        sl[:, :, 60:62] = yt.reshape(BL, COUT, 2, OW)
    return out


def make_in_maps(input_image: np.ndarray, weights: np.ndarray):
    V, Vt, U, Ut = _host_transforms(
        np.ascontiguousarray(input_image, dtype=np.float32),
        np.ascontiguousarray(weights, dtype=np.float32),
    )
    return [
        {
            "v_in": V[i * BL : (i + 1) * BL],
            "vt_in": np.ascontiguousarray(
                Vt[i * BL : (i + 1) * BL].transpose(1, 0, 2, 3)
            ),
            "w_t": U,
            "wt_t": Ut,
        }
        for i in range(N_CORES)
    ]


def kernel(input_image: np.ndarray, weights: np.ndarray) -> np.ndarray:
    in_maps = make_in_maps(input_image, weights)
    nc = _get_module()
    res = run_bass_kernel_spmd(nc, in_maps, list(range(N_CORES))).results
    return _host_combine(
        [r["m_out"] for r in res], [r["mt_out"] for r in res]
    )
